# revision 8
# baseline (speedup 1.0000x reference)
"""Trainium2 Bass kernel for BertForSpanAspectExtraction span scoring.

Computes, for x = sequence_output [B=4, L=256, H=768]:
  start_logits = x @ w_start + b_start                      [B, L]
  end_logits   = x @ w_end   + b_end                        [B, L]
  span_sig     = sigmoid(relu(s_i + e_j + b1) @ W2 + b2)    [B, L, L]
with s = x @ W1s, e = x @ W1e  (H2 = 384).

Sharding: 8 cores = (b, i-half).  Each core receives a column-rotated x[b].T
so its own 128 start-rows are always columns 0..127 (uniform SPMD program);
the host un-rotates the span columns after gathering.

On-chip per core:
  - PE: e_T [384,256] and s_T [384,128] projections (k on partitions).
  - Grid: per (i, kblock) one fused op h = relu(e_T + s_col) ([128,256],
    split between ScalarE-activation-bias and VectorE-tensor_scalar), then a
    K=128 matmul vs the W2 column reducing over k into a PSUM row [1,256]
    at col-group partition 32*(i%4) (4-way PE col-group concurrency).
  - PSUM layout packs 32 rows per [128,2048] tile; Sigmoid evacuates each
    bank [128,512] in one ScalarE op (8 span rows per op).
"""

import sys

if "/opt/trn_rl_repo" not in sys.path:
    sys.path.insert(0, "/opt/trn_rl_repo")

import numpy as np

B, L, H = 4, 256, 768
H2 = 384
NCORES = 8
LH = 128  # start-rows per core
NH = H // 128  # 6 contraction blocks
NK = H2 // 128  # 3 k blocks

# number of the 384 grid ops on ScalarE (rest on VectorE); tuned from profile
ACT_OPS = 108
H_BUFS = 16

_built = {}


def _build():
    import concourse.bacc as bacc
    import concourse.mybir as mybir
    import concourse.tile as tile
    from concourse.tile_rust import add_dep_helper

    f32 = mybir.dt.float32
    AF = mybir.ActivationFunctionType
    OP = mybir.AluOpType

    nc = bacc.Bacc("TRN2", debug=False, target_bir_lowering=False)

    xT = nc.dram_tensor("xT", [H, L], f32, kind="ExternalInput").ap()
    W1e = nc.dram_tensor("W1e", [H, H2], f32, kind="ExternalInput").ap()
    W1s = nc.dram_tensor("W1s", [H, H2], f32, kind="ExternalInput").ap()
    b1c = nc.dram_tensor("b1c", [H2, 1], f32, kind="ExternalInput").ap()
    W2r = nc.dram_tensor("W2r", [H2, 32], f32, kind="ExternalInput").ap()
    wsc = nc.dram_tensor("wsc", [H, 1], f32, kind="ExternalInput").ap()
    wec = nc.dram_tensor("wec", [H, 1], f32, kind="ExternalInput").ap()
    b2c = nc.dram_tensor("b2c", [128, 1], f32, kind="ExternalInput").ap()

    span = nc.dram_tensor("span", [LH, L], f32, kind="ExternalOutput").ap()
    slog = nc.dram_tensor("slog", [LH], f32, kind="ExternalOutput").ap()
    elog = nc.dram_tensor("elog", [LH], f32, kind="ExternalOutput").ap()

    with tile.TileContext(nc) as tc:
        with (
            tc.tile_pool(name="persist", bufs=1) as pp,
            tc.tile_pool(name="hpool", bufs=H_BUFS) as hp,
            tc.tile_pool(name="psum", bufs=2, space="PSUM") as pq,
            tc.tile_pool(name="stage", bufs=2) as sp,
        ):
            # ---------------- input loads ----------------
            xt = [pp.tile([128, L], f32, tag=f"xt{hb}", name=f"xt{hb}") for hb in range(NH)]
            for hb in range(NH):
                nc.sync.dma_start(xt[hb][:], xT[hb * 128 : (hb + 1) * 128, :])

            w1e = [pp.tile([128, H2], f32, tag=f"w1e{hb}", name=f"w1e{hb}") for hb in range(NH)]
            w1s = [pp.tile([128, H2], f32, tag=f"w1s{hb}", name=f"w1s{hb}") for hb in range(NH)]
            # per-kblock chunks so low-k columns land early
            for kb in range(NK):
                for hb in range(NH):
                    nc.sync.dma_start(
                        w1e[hb][:, kb * 128 : (kb + 1) * 128],
                        W1e[hb * 128 : (hb + 1) * 128, kb * 128 : (kb + 1) * 128],
                    )
            for kb in range(NK):
                for hb in range(NH):
                    nc.sync.dma_start(
                        w1s[hb][:, kb * 128 : (kb + 1) * 128],
                        W1s[hb * 128 : (hb + 1) * 128, kb * 128 : (kb + 1) * 128],
                    )

            b1sb = [pp.tile([128, 1], f32, tag=f"b1_{kb}", name=f"b1_{kb}") for kb in range(NK)]
            w2sb = [pp.tile([128, 32], f32, tag=f"w2_{kb}", name=f"w2_{kb}") for kb in range(NK)]
            for kb in range(NK):
                nc.sync.dma_start(b1sb[kb][:], b1c[kb * 128 : (kb + 1) * 128, :])
                nc.sync.dma_start(w2sb[kb][:], W2r[kb * 128 : (kb + 1) * 128, :])
            wssb = [pp.tile([128, 1], f32, tag=f"ws_{hb}", name=f"ws_{hb}") for hb in range(NH)]
            wesb = [pp.tile([128, 1], f32, tag=f"we_{hb}", name=f"we_{hb}") for hb in range(NH)]
            for hb in range(NH):
                nc.sync.dma_start(wssb[hb][:], wsc[hb * 128 : (hb + 1) * 128, :])
                nc.sync.dma_start(wesb[hb][:], wec[hb * 128 : (hb + 1) * 128, :])
            b2sb = pp.tile([128, 1], f32, tag="b2")
            nc.sync.dma_start(b2sb[:], b2c[:])

            # ---------------- projections ----------------
            eT = [pp.tile([128, L], f32, tag=f"eT{kb}", name=f"eT{kb}") for kb in range(NK)]
            sT = [pp.tile([128, LH], f32, tag=f"sT{kb}", name=f"sT{kb}") for kb in range(NK)]
            for kb in range(NK):
                pe = pq.tile([128, 2048], f32, tag="psum")
                for hb in range(NH):
                    nc.tensor.matmul(
                        pe[:, 0:L],
                        w1e[hb][:, kb * 128 : (kb + 1) * 128],
                        xt[hb][:],
                        start=(hb == 0),
                        stop=(hb == NH - 1),
                    )
                nc.scalar.activation(eT[kb][:], pe[:, 0:L], AF.Identity, bias=0.0)
            for kb in range(NK):
                ps = pq.tile([128, 2048], f32, tag="psum")
                for hb in range(NH):
                    nc.tensor.matmul(
                        ps[:, 0:LH],
                        w1s[hb][:, kb * 128 : (kb + 1) * 128],
                        xt[hb][:, 0:LH],
                        start=(hb == 0),
                        stop=(hb == NH - 1),
                    )
                # fold b1 into the mandatory PSUM->SBUF evacuation
                nc.scalar.activation(
                    sT[kb][:], ps[:, 0:LH], AF.Identity, bias=b1sb[kb][:]
                )

            # ---------------- span grid ----------------
            # local row i = 32*batch + m*4 + cg lives at PSUM partition 32*cg,
            # free offset m*256 (bank = m>>1)
            opct = 0
            n_grid_ops = LH * NK
            dstv = span.rearrange("(bt m cg) j -> bt m cg j", bt=LH // 32, m=8, cg=4)
            for batch in range(LH // 32):
                pt = pq.tile([128, 2048], f32, tag="psum")
                st = sp.tile([128, 2048], f32, tag="stage")
                prev_stop = [None] * 4  # last stop-matmul per cg (bank sharing)
                for m in range(8):
                    cur_stop = [None] * 4
                    for kb in range(NK):
                        for cg in range(4):
                            i = batch * 32 + m * 4 + cg
                            h = hp.tile([128, L], f32, tag="h")
                            use_act = (opct * ACT_OPS) % n_grid_ops < ACT_OPS
                            opct += 1
                            if use_act:
                                nc.scalar.activation(
                                    h[:], eT[kb][:], AF.Relu, bias=sT[kb][:, i : i + 1]
                                )
                            else:
                                nc.vector.tensor_scalar(
                                    h[:],
                                    eT[kb][:],
                                    sT[kb][:, i : i + 1],
                                    0.0,
                                    op0=OP.add,
                                    op1=OP.max,
                                )
                            off = m * 256
                            mm = nc.tensor.matmul(
                                pt[32 * cg : 32 * cg + 32, off : off + 256],
                                w2sb[kb][:],
                                h[:],
                                start=(kb == 0),
                                stop=(kb == NK - 1),
                                tile_position=(0, 32 * cg),
                                # sim's group-check mis-maps partition-offset
                                # outputs (tile pitch != 16KB); the functional
                                # pending-zero model is correct
                                skip_group_check=True,
                            )
                            # slots 2k/2k+1 share a PSUM bank: the start
                            # matmul (whole-zero-region has_written clear)
                            # must not reorder before the prior slot's stop
                            if kb == 0 and m % 2 == 1 and prev_stop[cg] is not None:
                                add_dep_helper(
                                    mm.ins,
                                    prev_stop[cg].ins,
                                    sync=False,
                                    reason="psum zero-region group ordering",
                                )
                            if kb == NK - 1:
                                cur_stop[cg] = mm
                    prev_stop = cur_stop
                for bank in range(4):
                    nc.scalar.activation(
                        st[:, bank * 512 : (bank + 1) * 512],
                        pt[:, bank * 512 : (bank + 1) * 512],
                        AF.Sigmoid,
                        bias=b2sb[:],
                    )
                for cg in range(4):
                    src = st[32 * cg : 32 * cg + 1, :].rearrange(
                        "p (m j) -> p m j", m=8
                    )
                    nc.sync.dma_start(dstv[batch, :, cg, :], src)

            # ---------------- start/end logits ----------------
            for wsb, outdram, tagn in ((wssb, slog, "sl"), (wesb, elog, "el")):
                pl = pq.tile([1, 128], f32, tag="psum")
                for hb in range(NH):
                    nc.tensor.matmul(
                        pl[:],
                        wsb[hb][:],
                        xt[hb][:, 0:LH],
                        start=(hb == 0),
                        stop=(hb == NH - 1),
                    )
                lt = sp.tile([1, 128], f32, tag="lt")
                nc.vector.tensor_copy(lt[:], pl[:])
                nc.sync.dma_start(outdram[:], lt[:])

    nc.compile()
    return nc


def _get_nc():
    if "nc" not in _built:
        _built["nc"] = _build()
    return _built["nc"]


def _make_in_maps(
    sequence_output, w_start, b_start, w_end, b_end, W1s, W1e, b1, W2, b2
):
    x = np.asarray(sequence_output, dtype=np.float32)
    shared = {
        "W1e": np.ascontiguousarray(np.asarray(W1e, np.float32)),
        "W1s": np.ascontiguousarray(np.asarray(W1s, np.float32)),
        "b1c": np.ascontiguousarray(np.asarray(b1, np.float32).reshape(H2, 1)),
        "W2r": np.ascontiguousarray(np.repeat(np.asarray(W2, np.float32).reshape(H2, 1), 32, axis=1)),
        "wsc": np.ascontiguousarray(np.asarray(w_start, np.float32).reshape(H, 1)),
        "wec": np.ascontiguousarray(np.asarray(w_end, np.float32).reshape(H, 1)),
        "b2c": np.full((128, 1), float(np.asarray(b2)), np.float32),
    }
    in_maps = []
    for core in range(NCORES):
        b, ih = core // 2, core % 2
        xTb = x[b].T  # [H, L]
        xTr = np.roll(xTb, -ih * LH, axis=1)  # own rows at cols 0..127
        m = dict(shared)
        m["xT"] = np.ascontiguousarray(xTr)
        in_maps.append(m)
    return in_maps


def kernel(
    sequence_output, w_start, b_start, w_end, b_end, W1s, W1e, b1, W2, b2
):
    from concourse.bass_utils import run_bass_kernel_spmd

    nc = _get_nc()
    in_maps = _make_in_maps(
        sequence_output, w_start, b_start, w_end, b_end, W1s, W1e, b1, W2, b2
    )
    res = run_bass_kernel_spmd(nc, in_maps, core_ids=list(range(NCORES)))
    span = np.empty((B, L, L), np.float32)
    sl = np.empty((B, L), np.float32)
    el = np.empty((B, L), np.float32)
    bs = float(np.asarray(b_start))
    be = float(np.asarray(b_end))
    for core in range(NCORES):
        b, ih = core // 2, core % 2
        r = res.results[core]
        span[b, ih * LH : (ih + 1) * LH, :] = np.roll(r["span"], ih * LH, axis=1)
        sl[b, ih * LH : (ih + 1) * LH] = r["slog"] + bs
        el[b, ih * LH : (ih + 1) * LH] = r["elog"] + be
    return (sl, el, span)


# revision 9
# speedup vs baseline: 1.3718x; 1.3718x over previous
"""Trainium2 Bass kernel for BertForSpanAspectExtraction span scoring.

Computes, for x = sequence_output [B=4, L=256, H=768]:
  start_logits = x @ w_start + b_start                      [B, L]
  end_logits   = x @ w_end   + b_end                        [B, L]
  span_sig     = sigmoid(relu(s_i + e_j + b1) @ W2 + b2)    [B, L, L]
with s = x @ W1s, e = x @ W1e  (H2 = 384).

Sharding: 8 cores = (b, i-half).  Each core receives a column-rotated x[b].T
so its own 128 start-rows are always columns 0..127 (uniform SPMD program);
the host un-rotates the span columns after gathering.

On-chip per core:
  - PE: e_T [384,256] and s_T [384,128] projections (k on partitions).
  - Grid: per (i, kblock) one fused op h = relu(e_T + s_col) ([128,256],
    split between ScalarE-activation-bias and VectorE-tensor_scalar), then a
    K=128 matmul vs the W2 column reducing over k into a PSUM row [1,256]
    at col-group partition 32*(i%4) (4-way PE col-group concurrency).
  - PSUM layout packs 32 rows per [128,2048] tile; Sigmoid evacuates each
    bank [128,512] in one ScalarE op (8 span rows per op).
"""

import sys

if "/opt/trn_rl_repo" not in sys.path:
    sys.path.insert(0, "/opt/trn_rl_repo")

import ml_dtypes
import numpy as np

B, L, H = 4, 256, 768
H2 = 384
NCORES = 8
LH = 128  # start-rows per core
NH = H // 128  # 6 contraction blocks
NK = H2 // 128  # 3 k blocks

# number of the 384 grid ops on ScalarE (rest on VectorE); tuned from profile
ACT_OPS = 91
H_BUFS = 16

_built = {}


def _build():
    import concourse.bacc as bacc
    import concourse.mybir as mybir
    import concourse.tile as tile
    from concourse.tile_rust import add_dep_helper

    f32 = mybir.dt.float32
    bf16 = mybir.dt.bfloat16
    AF = mybir.ActivationFunctionType
    OP = mybir.AluOpType

    nc = bacc.Bacc("TRN2", debug=False, target_bir_lowering=False)

    xT = nc.dram_tensor("xT", [H, L], f32, kind="ExternalInput").ap()
    W1e = nc.dram_tensor("W1e", [H, H2], bf16, kind="ExternalInput").ap()
    W1s = nc.dram_tensor("W1s", [H, H2], bf16, kind="ExternalInput").ap()
    b1c = nc.dram_tensor("b1c", [H2, 1], f32, kind="ExternalInput").ap()
    W2r = nc.dram_tensor("W2r", [H2, 32], bf16, kind="ExternalInput").ap()
    wsc = nc.dram_tensor("wsc", [H, 1], f32, kind="ExternalInput").ap()
    wec = nc.dram_tensor("wec", [H, 1], f32, kind="ExternalInput").ap()
    b2c = nc.dram_tensor("b2c", [128, 1], f32, kind="ExternalInput").ap()

    span = nc.dram_tensor("span", [LH, L], f32, kind="ExternalOutput").ap()
    slog = nc.dram_tensor("slog", [LH], f32, kind="ExternalOutput").ap()
    elog = nc.dram_tensor("elog", [LH], f32, kind="ExternalOutput").ap()

    with tile.TileContext(nc) as tc:
        with (
            tc.tile_pool(name="persist", bufs=1) as pp,
            tc.tile_pool(name="hpool", bufs=H_BUFS) as hp,
            tc.tile_pool(name="psum", bufs=2, space="PSUM") as pq,
            tc.tile_pool(name="stage", bufs=2) as sp,
        ):
            # ---------------- input loads ----------------
            xt = [pp.tile([128, L], f32, tag=f"xt{hb}", name=f"xt{hb}") for hb in range(NH)]
            for hb in range(NH):
                nc.sync.dma_start(xt[hb][:], xT[hb * 128 : (hb + 1) * 128, :])

            w1e = [pp.tile([128, H2], bf16, tag=f"w1e{hb}", name=f"w1e{hb}") for hb in range(NH)]
            w1s = [pp.tile([128, H2], bf16, tag=f"w1s{hb}", name=f"w1s{hb}") for hb in range(NH)]
            # whole-tile contiguous loads, spread across the ACT HWDGE ring and
            # the gpsimd SWDGE so they run parallel to the xT loads on the SP ring
            for hb in range(NH):
                nc.scalar.dma_start(w1e[hb][:], W1e[hb * 128 : (hb + 1) * 128, :])
            for hb in range(NH):
                nc.gpsimd.dma_start(w1s[hb][:], W1s[hb * 128 : (hb + 1) * 128, :])
            # bf16 copies of xT for the (all-bf16) projection matmuls
            xtb = [pp.tile([128, L], bf16, tag=f"xtb{hb}", name=f"xtb{hb}") for hb in range(NH)]
            for hb in range(NH):
                nc.vector.tensor_copy(xtb[hb][:], xt[hb][:])

            b1sb = [pp.tile([128, 1], f32, tag=f"b1_{kb}", name=f"b1_{kb}") for kb in range(NK)]
            w2sb = [pp.tile([128, 32], bf16, tag=f"w2_{kb}", name=f"w2_{kb}") for kb in range(NK)]
            for kb in range(NK):
                nc.gpsimd.dma_start(b1sb[kb][:], b1c[kb * 128 : (kb + 1) * 128, :])
                nc.gpsimd.dma_start(w2sb[kb][:], W2r[kb * 128 : (kb + 1) * 128, :])
            wssb = [pp.tile([128, 1], f32, tag=f"ws_{hb}", name=f"ws_{hb}") for hb in range(NH)]
            wesb = [pp.tile([128, 1], f32, tag=f"we_{hb}", name=f"we_{hb}") for hb in range(NH)]
            for hb in range(NH):
                nc.gpsimd.dma_start(wssb[hb][:], wsc[hb * 128 : (hb + 1) * 128, :])
                nc.gpsimd.dma_start(wesb[hb][:], wec[hb * 128 : (hb + 1) * 128, :])
            b2sb = pp.tile([128, 1], f32, tag="b2")
            nc.gpsimd.dma_start(b2sb[:], b2c[:])

            # ---------------- projections ----------------
            eT = [pp.tile([128, L], bf16, tag=f"eT{kb}", name=f"eT{kb}") for kb in range(NK)]
            sT = [pp.tile([128, LH], f32, tag=f"sT{kb}", name=f"sT{kb}") for kb in range(NK)]
            for kb in range(NK):
                pe = pq.tile([128, 2048], f32, tag="psum")
                for hb in range(NH):
                    nc.tensor.matmul(
                        pe[:, 0:L],
                        w1e[hb][:, kb * 128 : (kb + 1) * 128],
                        xtb[hb][:],
                        start=(hb == 0),
                        stop=(hb == NH - 1),
                    )
                nc.scalar.activation(eT[kb][:], pe[:, 0:L], AF.Identity, bias=0.0)
            for kb in range(NK):
                ps = pq.tile([128, 2048], f32, tag="psum")
                for hb in range(NH):
                    nc.tensor.matmul(
                        ps[:, 0:LH],
                        w1s[hb][:, kb * 128 : (kb + 1) * 128],
                        xtb[hb][:, 0:LH],
                        start=(hb == 0),
                        stop=(hb == NH - 1),
                    )
                # fold b1 into the mandatory PSUM->SBUF evacuation
                nc.scalar.activation(
                    sT[kb][:], ps[:, 0:LH], AF.Identity, bias=b1sb[kb][:]
                )

            # ---------------- span grid ----------------
            # local row i = 32*batch + m*4 + cg lives at PSUM partition 32*cg,
            # free offset m*256 (bank = m>>1)
            opct = 0
            n_grid_ops = LH * NK
            dstv = span.rearrange("(bt m cg) j -> bt m cg j", bt=LH // 32, m=8, cg=4)
            for batch in range(LH // 32):
                pt = pq.tile([128, 2048], f32, tag="psum")
                st = sp.tile([128, 2048], f32, tag="stage")
                prev_stop = [None] * 4  # last stop-matmul per cg (bank sharing)
                for m in range(8):
                    cur_stop = [None] * 4
                    for kb in range(NK):
                        for cg in range(4):
                            i = batch * 32 + m * 4 + cg
                            h = hp.tile([128, L], bf16, tag="h")
                            use_act = (opct * ACT_OPS) % n_grid_ops < ACT_OPS
                            opct += 1
                            if use_act:
                                nc.scalar.activation(
                                    h[:], eT[kb][:], AF.Relu, bias=sT[kb][:, i : i + 1]
                                )
                            else:
                                nc.vector.tensor_scalar(
                                    h[:],
                                    eT[kb][:],
                                    sT[kb][:, i : i + 1],
                                    0.0,
                                    op0=OP.add,
                                    op1=OP.max,
                                )
                            off = m * 256
                            mm = nc.tensor.matmul(
                                pt[32 * cg : 32 * cg + 32, off : off + 256],
                                w2sb[kb][:],
                                h[:],
                                start=(kb == 0),
                                stop=(kb == NK - 1),
                                tile_position=(0, 32 * cg),
                                # sim's group-check mis-maps partition-offset
                                # outputs (tile pitch != 16KB); the functional
                                # pending-zero model is correct
                                skip_group_check=True,
                            )
                            # slots 2k/2k+1 share a PSUM bank: the start
                            # matmul (whole-zero-region has_written clear)
                            # must not reorder before the prior slot's stop
                            if kb == 0 and m % 2 == 1 and prev_stop[cg] is not None:
                                add_dep_helper(
                                    mm.ins,
                                    prev_stop[cg].ins,
                                    sync=False,
                                    reason="psum zero-region group ordering",
                                )
                            if kb == NK - 1:
                                cur_stop[cg] = mm
                    prev_stop = cur_stop
                for half in range(2):
                    nc.scalar.activation(
                        st[:, half * 1024 : (half + 1) * 1024],
                        pt[:, half * 1024 : (half + 1) * 1024],
                        AF.Sigmoid,
                        bias=b2sb[:],
                    )
                for cg in range(4):
                    src = st[32 * cg : 32 * cg + 1, :].rearrange(
                        "p (m j) -> p m j", m=8
                    )
                    nc.sync.dma_start(dstv[batch, :, cg, :], src)

            # ---------------- start/end logits ----------------
            for wsb, outdram, tagn in ((wssb, slog, "sl"), (wesb, elog, "el")):
                pl = pq.tile([1, 128], f32, tag="psum")
                for hb in range(NH):
                    nc.tensor.matmul(
                        pl[:],
                        wsb[hb][:],
                        xt[hb][:, 0:LH],
                        start=(hb == 0),
                        stop=(hb == NH - 1),
                    )
                lt = sp.tile([1, 128], f32, tag="lt")
                nc.vector.tensor_copy(lt[:], pl[:])
                nc.sync.dma_start(outdram[:], lt[:])

    nc.compile()
    return nc


def _get_nc():
    if "nc" not in _built:
        _built["nc"] = _build()
    return _built["nc"]


def _make_in_maps(
    sequence_output, w_start, b_start, w_end, b_end, W1s, W1e, b1, W2, b2
):
    x = np.asarray(sequence_output, dtype=np.float32)
    shared = {
        "W1e": np.ascontiguousarray(np.asarray(W1e, np.float32)).astype(ml_dtypes.bfloat16),
        "W1s": np.ascontiguousarray(np.asarray(W1s, np.float32)).astype(ml_dtypes.bfloat16),
        "b1c": np.ascontiguousarray(np.asarray(b1, np.float32).reshape(H2, 1)),
        "W2r": np.ascontiguousarray(np.repeat(np.asarray(W2, np.float32).reshape(H2, 1), 32, axis=1)).astype(ml_dtypes.bfloat16),
        "wsc": np.ascontiguousarray(np.asarray(w_start, np.float32).reshape(H, 1)),
        "wec": np.ascontiguousarray(np.asarray(w_end, np.float32).reshape(H, 1)),
        "b2c": np.full((128, 1), float(np.asarray(b2)), np.float32),
    }
    in_maps = []
    for core in range(NCORES):
        b, ih = core // 2, core % 2
        xTb = x[b].T  # [H, L]
        xTr = np.roll(xTb, -ih * LH, axis=1)  # own rows at cols 0..127
        m = dict(shared)
        m["xT"] = np.ascontiguousarray(xTr)
        in_maps.append(m)
    return in_maps


def kernel(
    sequence_output, w_start, b_start, w_end, b_end, W1s, W1e, b1, W2, b2
):
    from concourse.bass_utils import run_bass_kernel_spmd

    nc = _get_nc()
    in_maps = _make_in_maps(
        sequence_output, w_start, b_start, w_end, b_end, W1s, W1e, b1, W2, b2
    )
    res = run_bass_kernel_spmd(nc, in_maps, core_ids=list(range(NCORES)))
    span = np.empty((B, L, L), np.float32)
    sl = np.empty((B, L), np.float32)
    el = np.empty((B, L), np.float32)
    bs = float(np.asarray(b_start))
    be = float(np.asarray(b_end))
    for core in range(NCORES):
        b, ih = core // 2, core % 2
        r = res.results[core]
        span[b, ih * LH : (ih + 1) * LH, :] = np.roll(r["span"], ih * LH, axis=1)
        sl[b, ih * LH : (ih + 1) * LH] = r["slog"] + bs
        el[b, ih * LH : (ih + 1) * LH] = r["elog"] + be
    return (sl, el, span)
